# revision 17
# baseline (speedup 1.0000x reference)
"""Distributed Trainium2 kernel for AnomalyMoE k-NN retrieval.

reference:  q = l2norm(test[L,N,D]); g = l2norm(normal[L,M,D])
            sim[l,n,m] = q . g ; out = (1 - mean_l max_m sim).reshape(1,1,16,16)

Strategy (8 NeuronCores):
- Shard gallery along M (6400 rows/core). Host pre-transposes each shard to
  [L, D, M_shard] (fp8e4m3 by default) so every device DMA is dense; queries
  ship in both layouts ([L,D,N] for matmul weights, [L,N,D] for norms).
- Per core: dot[n,m] accumulated on TensorE with fp8 DoubleRow pair-matmuls
  (contraction 256/instruction).  Gallery row norms via ACT Square +
  ones-DoubleRow-matmul (broadcasts colsums to all 128 partitions), then
  one ACT Abs_reciprocal_sqrt.  Queries are NOT normalized on the way in:
  1/||q_n|| is applied to the per-layer maxes at the end (positive
  per-query scale commutes with max over gallery).
- local layer-max via DVE mult + running max; AllReduce(max) over 8 cores
  on [128, 8] f32; final mean over layers, 1-x, output [2,128] f32.
"""

import os
import sys
from concurrent.futures import ThreadPoolExecutor

sys.path.insert(0, "/opt/trn_rl_repo")

import numpy as np
import ml_dtypes

import concourse.bacc as bacc
import concourse.mybir as mybir
import concourse.tile as tile
from concourse.bass_utils import run_bass_kernel_spmd

F32 = mybir.dt.float32
BF16 = mybir.dt.bfloat16
AF = mybir.ActivationFunctionType
DR = mybir.MatmulPerfMode.DoubleRow

MODE = os.environ.get("KERNEL_MODE", "fp8")  # "fp8" | "bf16"
if MODE == "fp8":
    DT_IN = mybir.dt.float8e4
    NP_IN = ml_dtypes.float8_e4m3fn
else:
    DT_IN = mybir.dt.bfloat16
    NP_IN = ml_dtypes.bfloat16

NCORES = 8
L = 4
D = 1024
N = 256
M_FULL = 51200
MS = M_FULL // NCORES  # 6400 per core
KC = D // 128  # 8 contraction chunks of 128
KP = KC // 2  # 4 DoubleRow pairs
SUPER = 512
SUPERS = [(m0, min(SUPER, MS - m0)) for m0 in range(0, MS, SUPER)]  # 12x512 + 256
DVE_SQ_MOD = int(os.environ.get("KERNEL_DVE_SQ_MOD", "0"))  # 0 = all squares on ACT
SKEW = int(os.environ.get("KERNEL_SKEW", "1"))  # software-pipeline depth
NORM_DMA = os.environ.get("KERNEL_NORM_DMA", "0") == "1"
SQ_SPLIT = int(os.environ.get("KERNEL_SQ_SPLIT", "0"))  # k-chunks squared on DVE
KERNEL_TAG = os.environ.get("KERNEL_TAG", "")
NEG = -3.0e38


def build():
    nc = bacc.Bacc("TRN2", target_bir_lowering=False, debug=False, num_devices=NCORES)
    g_ext = nc.dram_tensor("g_t", [L, D, MS], DT_IN, kind="ExternalInput")
    qt_ext = nc.dram_tensor("q_t", [L, D, N], DT_IN, kind="ExternalInput")
    qn_ext = nc.dram_tensor("q_n", [L, N, D], DT_IN, kind="ExternalInput")
    out_ext = nc.dram_tensor("out", [2, 128], F32, kind="ExternalOutput")

    cc_in = nc.dram_tensor("cc_in", [128, 2 * L], F32)
    cc_out = nc.dram_tensor("cc_out", [128, 2 * L], F32, addr_space="Shared")

    with tile.TileContext(nc) as tc:
        with (
            tc.tile_pool(name="persist", bufs=1) as pp,
            tc.tile_pool(name="gsup", bufs=4) as gpool,
            tc.tile_pool(name="sqp", bufs=4) as sqpool,
            tc.tile_pool(name="invgp", bufs=3) as invgpool,
            tc.tile_pool(name="simp", bufs=3) as simpool,
            tc.tile_pool(name="qsqp", bufs=2) as qsqpool,
            tc.tile_pool(name="pm0", bufs=3, space="PSUM") as pm0pool,
            tc.tile_pool(name="pm1", bufs=3, space="PSUM") as pm1pool,
            tc.tile_pool(name="pnorm", bufs=2, space="PSUM") as pnormpool,
        ):
            # ---- persistent tiles ----
            qt_sb = pp.tile([128, L * KC, N], DT_IN, name="qt_sb")
            nc.sync.dma_start(
                qt_sb[:], qt_ext.ap().rearrange("l (k p) n -> p (l k) n", p=128)
            )
            qn_sb = pp.tile([128, 2 * L, D], DT_IN, name="qn_sb")
            nc.sync.dma_start(
                qn_sb[:], qn_ext.ap().rearrange("l (c p) d -> p (l c) d", p=128)
            )
            if MODE == "fp8":
                ones_sb = pp.tile([128, 2, 128], DT_IN, name="ones_sb")
            else:
                ones_sb = pp.tile([128, 128], DT_IN, name="ones_sb")
            nc.gpsimd.memset(ones_sb[:], 1.0)
            runmax = pp.tile([128, 2 * L, SUPER], BF16, name="runmax")
            nc.gpsimd.memset(runmax[:], NEG)
            qss = pp.tile([128, 2 * L], F32, name="qss")
            invq = pp.tile([128, 2 * L], F32, name="invq")
            lmax_sb = pp.tile([128, 2 * L], F32, name="lmax_sb")
            gmax_sb = pp.tile([128, 2 * L], F32, name="gmax_sb")
            smax_sb = pp.tile([128, 2 * L], F32, name="smax_sb")
            res_sb = pp.tile([128, 2], F32, name="res_sb")
            if KERNEL_TAG:
                # cache-buster: changes the BIR so stale NEFF caches miss
                tag_sb = pp.tile([128, 1], F32, name=f"tag_{KERNEL_TAG}")
                nc.gpsimd.memset(tag_sb[:], 1.0)

            # ---- query squared norms: qss[:, l*2+c] = sum_d q[l, c*128+p, d]^2
            for j in range(2 * L):
                qsq_scr = qsqpool.tile([128, D], BF16, name="qsq_scr")
                nc.scalar.activation(
                    qsq_scr[:],
                    qn_sb[:, j, :],
                    AF.Square,
                    accum_out=qss[:, j : j + 1],
                )
            # invq = 1/sqrt(qss)
            nc.scalar.activation(invq[:], qss[:], AF.Abs_reciprocal_sqrt)

            # ---- main loop over layers and m-supers, software-pipelined ----
            # Stage A (dma + squares) runs SKEW supers ahead of stage B
            # (norm-mm, rsqrt, main-mms, epilogue) so the in-order ACT queue
            # never head-of-line-blocks a square behind an rsqrt.

            def stage_a(lx, m0, msz, sidx):
                gsup = gpool.tile([128, KC, SUPER], DT_IN, name="gsup")
                nc.sync.dma_start(
                    gsup[:, :, :msz],
                    g_ext.ap()[lx].rearrange("(k p) m -> p k m", p=128)[
                        :, :, m0 : m0 + msz
                    ],
                )
                sq = sqpool.tile([128, KC, SUPER], DT_IN, name="sq")
                if SQ_SPLIT:
                    # fixed per-super split: last SQ_SPLIT k-chunks on DVE
                    ka = KC - SQ_SPLIT
                    nc.scalar.activation(
                        sq[:, : ka // 2, :msz], gsup[:, : ka // 2, :msz], AF.Square
                    )
                    nc.scalar.activation(
                        sq[:, ka // 2 : ka, :msz],
                        gsup[:, ka // 2 : ka, :msz],
                        AF.Square,
                    )
                    nc.vector.tensor_tensor(
                        out=sq[:, ka:, :msz],
                        in0=gsup[:, ka:, :msz],
                        in1=gsup[:, ka:, :msz],
                        op=mybir.AluOpType.mult,
                    )
                    return gsup, sq
                on_dve = DVE_SQ_MOD and sidx % DVE_SQ_MOD == DVE_SQ_MOD - 1
                for h in (0, 1):
                    ksl = slice(h * KC // 2, (h + 1) * KC // 2)
                    if on_dve:
                        nc.vector.tensor_tensor(
                            out=sq[:, ksl, :msz],
                            in0=gsup[:, ksl, :msz],
                            in1=gsup[:, ksl, :msz],
                            op=mybir.AluOpType.mult,
                        )
                    else:
                        nc.scalar.activation(
                            sq[:, ksl, :msz], gsup[:, ksl, :msz], AF.Square
                        )
                return gsup, sq

            def stage_b(lx, m0, msz, gsup, sq):
                # gallery norms: pnorm[p, m] = sum_d g[d, m]^2 (all rows equal)
                pnorm = pnormpool.tile([128, SUPER], F32, name="pnorm")
                if MODE == "fp8" and NORM_DMA:
                    # fold k-chunks 8->2 with CCE-add DMAs (off the PE), then
                    # a single DoubleRow ones-matmul does the partition sum.
                    nc.gpsimd.dma_start(
                        out=sq[:, 0:4, :msz],
                        in_=sq[:, 4:8, :msz],
                        accum_op=mybir.AluOpType.add,
                    )
                    nc.gpsimd.dma_start(
                        out=sq[:, 0:2, :msz],
                        in_=sq[:, 2:4, :msz],
                        accum_op=mybir.AluOpType.add,
                    )
                    nc.tensor.matmul(
                        pnorm[:, :msz],
                        ones_sb[:],
                        sq[:, 0:2, :msz],
                        start=True,
                        stop=True,
                        perf_mode=DR,
                    )
                elif MODE == "fp8":
                    for j in range(KP):
                        nc.tensor.matmul(
                            pnorm[:, :msz],
                            ones_sb[:],
                            sq[:, 2 * j : 2 * j + 2, :msz],
                            start=(j == 0),
                            stop=(j == KP - 1),
                            perf_mode=DR,
                        )
                else:
                    for k in range(KC):
                        nc.tensor.matmul(
                            pnorm[:, :msz],
                            ones_sb[:],
                            sq[:, k, :msz],
                            start=(k == 0),
                            stop=(k == KC - 1),
                        )
                # invg = 1/sqrt(pnorm) on ACT
                invg = invgpool.tile([128, SUPER], F32, name="invg")
                nc.scalar.activation(
                    invg[:, :msz], pnorm[:, :msz], AF.Abs_reciprocal_sqrt
                )

                for cx, pmpool in ((0, pm0pool), (1, pm1pool)):
                    pm = pmpool.tile([128, SUPER], F32, name=f"pm{cx}")
                    if MODE == "fp8":
                        for j in range(KP):
                            nc.tensor.matmul(
                                pm[:, :msz],
                                qt_sb[
                                    :,
                                    lx * KC + 2 * j : lx * KC + 2 * j + 2,
                                    cx * 128 : (cx + 1) * 128,
                                ],
                                gsup[:, 2 * j : 2 * j + 2, :msz],
                                start=(j == 0),
                                stop=(j == KP - 1),
                                perf_mode=DR,
                            )
                    else:
                        for k in range(KC):
                            nc.tensor.matmul(
                                pm[:, :msz],
                                qt_sb[:, lx * KC + k, cx * 128 : (cx + 1) * 128],
                                gsup[:, k, :msz],
                                start=(k == 0),
                                stop=(k == KC - 1),
                            )
                    sim = simpool.tile([128, SUPER], BF16, name="sim")
                    nc.vector.tensor_tensor(
                        out=sim[:, :msz],
                        in0=pm[:, :msz],
                        in1=invg[:, :msz],
                        op=mybir.AluOpType.mult,
                    )
                    j2 = lx * 2 + cx
                    nc.vector.tensor_tensor(
                        out=runmax[:, j2, :msz],
                        in0=runmax[:, j2, :msz],
                        in1=sim[:, :msz],
                        op=mybir.AluOpType.max,
                    )

            work = [
                (lx, m0, msz) for lx in range(L) for (m0, msz) in SUPERS
            ]
            pending = []
            for sidx, (lx, m0, msz) in enumerate(work):
                pending.append((lx, m0, msz) + stage_a(lx, m0, msz, sidx))
                if len(pending) > SKEW:
                    stage_b(*pending.pop(0))
            while pending:
                stage_b(*pending.pop(0))

            # ---- local per-(layer, chunk) max ----
            for j in range(2 * L):
                nc.vector.reduce_max(
                    lmax_sb[:, j : j + 1], runmax[:, j, :], axis=mybir.AxisListType.X
                )

            # ---- all-reduce max over the 8 cores ----
            nc.gpsimd.dma_start(cc_in.ap(), lmax_sb[:])
            nc.gpsimd.collective_compute(
                "AllReduce",
                mybir.AluOpType.max,
                replica_groups=[list(range(NCORES))],
                ins=[cc_in.ap().opt()],
                outs=[cc_out.ap().opt()],
            )
            nc.gpsimd.dma_start(gmax_sb[:], cc_out.ap())

            # ---- scale by 1/||q||, mean over layers, 1 - x ----
            nc.vector.tensor_tensor(
                out=smax_sb[:], in0=gmax_sb[:], in1=invq[:], op=mybir.AluOpType.mult
            )
            for cx in range(2):
                ssum = pp.tile([128, 1], F32, name=f"ssum{cx}")
                nc.vector.reduce_sum(
                    ssum[:],
                    smax_sb[:, cx : 2 * L : 2],
                    axis=mybir.AxisListType.X,
                )
                # out = 1 - ssum/L
                nc.scalar.activation(
                    res_sb[:, cx : cx + 1], ssum[:], AF.Copy, bias=1.0, scale=-1.0 / L
                )
            for cx in range(2):
                nc.sync.dma_start(
                    out_ext.ap()[cx : cx + 1, :].rearrange("c p -> p c"),
                    res_sb[:, cx : cx + 1],
                )

    nc.compile()
    return nc


_NC_CACHE = None


def _get_nc():
    global _NC_CACHE
    if _NC_CACHE is None:
        _NC_CACHE = build()
    return _NC_CACHE


def _prep_shard(g_lp, c):
    # [L, MS, D] slice -> [L, D, MS] contiguous
    sl = g_lp[:, c * MS : (c + 1) * MS, :]
    return np.ascontiguousarray(sl.transpose(0, 2, 1))


def _prep_inputs(test_patch_tokens, normal_patch_tokens):
    q = np.asarray(test_patch_tokens, dtype=np.float32)
    g = np.asarray(normal_patch_tokens, dtype=np.float32)
    qn_lp = q.astype(NP_IN)  # [L, N, D]
    qt_lp = np.ascontiguousarray(qn_lp.transpose(0, 2, 1))  # [L, D, N]
    g_lp = g.astype(NP_IN)  # [L, M, D]
    with ThreadPoolExecutor(NCORES) as ex:
        shards = list(ex.map(lambda c: _prep_shard(g_lp, c), range(NCORES)))
    return [
        {"g_t": shards[c], "q_t": qt_lp, "q_n": qn_lp} for c in range(NCORES)
    ]


def kernel(test_patch_tokens: np.ndarray, normal_patch_tokens: np.ndarray):
    in_maps = _prep_inputs(test_patch_tokens, normal_patch_tokens)
    nc = _get_nc()
    results = run_bass_kernel_spmd(nc, in_maps, core_ids=list(range(NCORES))).results
    out = results[0]["out"].astype(np.float32).reshape(1, 1, 16, 16)
    return out


# revision 23
# speedup vs baseline: 1.5257x; 1.5257x over previous
"""Distributed Trainium2 kernel for AnomalyMoE k-NN retrieval.

reference:  q = l2norm(test[L,N,D]); g = l2norm(normal[L,M,D])
            sim[l,n,m] = q . g ; out = (1 - mean_l max_m sim).reshape(1,1,16,16)

Strategy (8 NeuronCores):
- Shard gallery along M (6400 rows/core). Host pre-transposes each shard to
  [L, D, M_shard] (fp8e4m3 by default) so every device DMA is dense; queries
  ship in both layouts ([L,D,N] for matmul weights, [L,N,D] for norms).
- Per core: dot[n,m] accumulated on TensorE with fp8 DoubleRow pair-matmuls
  (contraction 256/instruction).  Gallery row norms via ACT Square +
  ones-DoubleRow-matmul (broadcasts colsums to all 128 partitions), then
  one ACT Abs_reciprocal_sqrt.  Queries are NOT normalized on the way in:
  1/||q_n|| is applied to the per-layer maxes at the end (positive
  per-query scale commutes with max over gallery).
- local layer-max via DVE mult + running max; AllReduce(max) over 8 cores
  on [128, 8] f32; final mean over layers, 1-x, output [2,128] f32.
"""

import os
import sys
from concurrent.futures import ThreadPoolExecutor

sys.path.insert(0, "/opt/trn_rl_repo")

import numpy as np
import ml_dtypes

import concourse.bacc as bacc
import concourse.mybir as mybir
import concourse.tile as tile
from concourse.bass_utils import run_bass_kernel_spmd

F32 = mybir.dt.float32
BF16 = mybir.dt.bfloat16
AF = mybir.ActivationFunctionType
DR = mybir.MatmulPerfMode.DoubleRow

MODE = os.environ.get("KERNEL_MODE", "fp8")  # "fp8" | "bf16"
if MODE == "fp8":
    DT_IN = mybir.dt.float8e4
    NP_IN = ml_dtypes.float8_e4m3fn
else:
    DT_IN = mybir.dt.bfloat16
    NP_IN = ml_dtypes.bfloat16

NCORES = 8
L = 4
D = 1024
N = 256
M_FULL = 51200
MS = M_FULL // NCORES  # 6400 per core
KC = D // 128  # 8 contraction chunks of 128
KP = KC // 2  # 4 DoubleRow pairs
SUPER = 512
SUPERS = [(m0, min(SUPER, MS - m0)) for m0 in range(0, MS, SUPER)]  # 12x512 + 256
DVE_SQ_MOD = int(os.environ.get("KERNEL_DVE_SQ_MOD", "0"))  # 0 = all squares on ACT
SKEW = int(os.environ.get("KERNEL_SKEW", "1"))  # software-pipeline depth
NORM_DMA = os.environ.get("KERNEL_NORM_DMA", "0") == "1"
SQ_SPLIT = int(os.environ.get("KERNEL_SQ_SPLIT", "2"))  # k-chunks squared on DVE
HOST_COMBINE = os.environ.get("KERNEL_HOST_COMBINE", "0") == "1"
SPLIT_CC = os.environ.get("KERNEL_SPLIT_CC", "1") == "1"
KERNEL_TAG = os.environ.get("KERNEL_TAG", "")
NEG = -3.0e38


def build():
    nc = bacc.Bacc("TRN2", target_bir_lowering=False, debug=False, num_devices=NCORES)
    g_ext = nc.dram_tensor("g_t", [L, D, MS], DT_IN, kind="ExternalInput")
    qt_ext = nc.dram_tensor("q_t", [L, D, N], DT_IN, kind="ExternalInput")
    qn_ext = nc.dram_tensor("q_n", [L, N, D], DT_IN, kind="ExternalInput")
    if HOST_COMBINE:
        lmax_ext = nc.dram_tensor("out_lmax", [128, 2 * L], F32, kind="ExternalOutput")
        invq_ext = nc.dram_tensor("out_invq", [128, 2 * L], F32, kind="ExternalOutput")
        out_ext = cc_in = cc_out = None
    else:
        out_ext = nc.dram_tensor("out", [2, 128], F32, kind="ExternalOutput")
        cc_in = nc.dram_tensor("cc_in", [2 * L, 128], F32)
        cc_out = nc.dram_tensor("cc_out", [2 * L, 128], F32, addr_space="Shared")
        lmax_ext = invq_ext = None

    with tile.TileContext(nc) as tc:
        with (
            tc.tile_pool(name="persist", bufs=1) as pp,
            tc.tile_pool(name="gsup", bufs=4) as gpool,
            tc.tile_pool(name="sqp", bufs=4) as sqpool,
            tc.tile_pool(name="invgp", bufs=3) as invgpool,
            tc.tile_pool(name="simp", bufs=3) as simpool,
            tc.tile_pool(name="qsqp", bufs=2) as qsqpool,
            tc.tile_pool(name="pm0", bufs=3, space="PSUM") as pm0pool,
            tc.tile_pool(name="pm1", bufs=3, space="PSUM") as pm1pool,
            tc.tile_pool(name="pnorm", bufs=2, space="PSUM") as pnormpool,
        ):
            # ---- persistent tiles ----
            qt_sb = pp.tile([128, L * KC, N], DT_IN, name="qt_sb")
            nc.sync.dma_start(
                qt_sb[:], qt_ext.ap().rearrange("l (k p) n -> p (l k) n", p=128)
            )
            qn_sb = pp.tile([128, 2 * L, D], DT_IN, name="qn_sb")
            nc.sync.dma_start(
                qn_sb[:], qn_ext.ap().rearrange("l (c p) d -> p (l c) d", p=128)
            )
            if MODE == "fp8":
                ones_sb = pp.tile([128, 2, 128], DT_IN, name="ones_sb")
            else:
                ones_sb = pp.tile([128, 128], DT_IN, name="ones_sb")
            nc.gpsimd.memset(ones_sb[:], 1.0)
            runmax = pp.tile([128, 2 * L, SUPER], BF16, name="runmax")
            nc.gpsimd.memset(runmax[:], NEG)
            qss = pp.tile([128, 2 * L], F32, name="qss")
            invq = pp.tile([128, 2 * L], F32, name="invq")
            lmax_sb = pp.tile([128, 2 * L], F32, name="lmax_sb")
            gmax_sb = pp.tile([128, 2 * L], F32, name="gmax_sb")
            smax_sb = pp.tile([128, 2 * L], F32, name="smax_sb")
            res_sb = pp.tile([128, 2], F32, name="res_sb")
            if KERNEL_TAG:
                # cache-buster: changes the BIR so stale NEFF caches miss
                tag_sb = pp.tile([128, 1], F32, name=f"tag_{KERNEL_TAG}")
                nc.gpsimd.memset(tag_sb[:], 1.0)

            # ---- query squared norms: qss[:, l*2+c] = sum_d q[l, c*128+p, d]^2
            for j in range(2 * L):
                qsq_scr = qsqpool.tile([128, D], BF16, name="qsq_scr")
                nc.scalar.activation(
                    qsq_scr[:],
                    qn_sb[:, j, :],
                    AF.Square,
                    accum_out=qss[:, j : j + 1],
                )
            # invq = 1/sqrt(qss)
            nc.scalar.activation(invq[:], qss[:], AF.Abs_reciprocal_sqrt)

            # ---- main loop over layers and m-supers, software-pipelined ----
            # Stage A (dma + squares) runs SKEW supers ahead of stage B
            # (norm-mm, rsqrt, main-mms, epilogue) so the in-order ACT queue
            # never head-of-line-blocks a square behind an rsqrt.

            def stage_a(lx, m0, msz, sidx):
                gsup = gpool.tile([128, KC, SUPER], DT_IN, name="gsup")
                nc.sync.dma_start(
                    gsup[:, :, :msz],
                    g_ext.ap()[lx].rearrange("(k p) m -> p k m", p=128)[
                        :, :, m0 : m0 + msz
                    ],
                )
                sq = sqpool.tile([128, KC, SUPER], DT_IN, name="sq")
                if SQ_SPLIT:
                    # fixed per-super split: last SQ_SPLIT k-chunks on DVE
                    ka = KC - SQ_SPLIT
                    nc.scalar.activation(
                        sq[:, : ka // 2, :msz], gsup[:, : ka // 2, :msz], AF.Square
                    )
                    nc.scalar.activation(
                        sq[:, ka // 2 : ka, :msz],
                        gsup[:, ka // 2 : ka, :msz],
                        AF.Square,
                    )
                    nc.vector.tensor_tensor(
                        out=sq[:, ka:, :msz],
                        in0=gsup[:, ka:, :msz],
                        in1=gsup[:, ka:, :msz],
                        op=mybir.AluOpType.mult,
                    )
                    return gsup, sq
                on_dve = DVE_SQ_MOD and sidx % DVE_SQ_MOD == DVE_SQ_MOD - 1
                for h in (0, 1):
                    ksl = slice(h * KC // 2, (h + 1) * KC // 2)
                    if on_dve:
                        nc.vector.tensor_tensor(
                            out=sq[:, ksl, :msz],
                            in0=gsup[:, ksl, :msz],
                            in1=gsup[:, ksl, :msz],
                            op=mybir.AluOpType.mult,
                        )
                    else:
                        nc.scalar.activation(
                            sq[:, ksl, :msz], gsup[:, ksl, :msz], AF.Square
                        )
                return gsup, sq

            def stage_b(lx, m0, msz, gsup, sq):
                # gallery norms: pnorm[p, m] = sum_d g[d, m]^2 (all rows equal)
                pnorm = pnormpool.tile([128, SUPER], F32, name="pnorm")
                if MODE == "fp8" and NORM_DMA:
                    # fold k-chunks 8->2 with CCE-add DMAs (off the PE), then
                    # a single DoubleRow ones-matmul does the partition sum.
                    nc.gpsimd.dma_start(
                        out=sq[:, 0:4, :msz],
                        in_=sq[:, 4:8, :msz],
                        accum_op=mybir.AluOpType.add,
                    )
                    nc.gpsimd.dma_start(
                        out=sq[:, 0:2, :msz],
                        in_=sq[:, 2:4, :msz],
                        accum_op=mybir.AluOpType.add,
                    )
                    nc.tensor.matmul(
                        pnorm[:, :msz],
                        ones_sb[:],
                        sq[:, 0:2, :msz],
                        start=True,
                        stop=True,
                        perf_mode=DR,
                    )
                elif MODE == "fp8":
                    for j in range(KP):
                        nc.tensor.matmul(
                            pnorm[:, :msz],
                            ones_sb[:],
                            sq[:, 2 * j : 2 * j + 2, :msz],
                            start=(j == 0),
                            stop=(j == KP - 1),
                            perf_mode=DR,
                        )
                else:
                    for k in range(KC):
                        nc.tensor.matmul(
                            pnorm[:, :msz],
                            ones_sb[:],
                            sq[:, k, :msz],
                            start=(k == 0),
                            stop=(k == KC - 1),
                        )
                # invg = 1/sqrt(pnorm) on ACT
                invg = invgpool.tile([128, SUPER], F32, name="invg")
                nc.scalar.activation(
                    invg[:, :msz], pnorm[:, :msz], AF.Abs_reciprocal_sqrt
                )

                for cx, pmpool in ((0, pm0pool), (1, pm1pool)):
                    pm = pmpool.tile([128, SUPER], F32, name=f"pm{cx}")
                    if MODE == "fp8":
                        for j in range(KP):
                            nc.tensor.matmul(
                                pm[:, :msz],
                                qt_sb[
                                    :,
                                    lx * KC + 2 * j : lx * KC + 2 * j + 2,
                                    cx * 128 : (cx + 1) * 128,
                                ],
                                gsup[:, 2 * j : 2 * j + 2, :msz],
                                start=(j == 0),
                                stop=(j == KP - 1),
                                perf_mode=DR,
                            )
                    else:
                        for k in range(KC):
                            nc.tensor.matmul(
                                pm[:, :msz],
                                qt_sb[:, lx * KC + k, cx * 128 : (cx + 1) * 128],
                                gsup[:, k, :msz],
                                start=(k == 0),
                                stop=(k == KC - 1),
                            )
                    sim = simpool.tile([128, SUPER], BF16, name="sim")
                    nc.vector.tensor_tensor(
                        out=sim[:, :msz],
                        in0=pm[:, :msz],
                        in1=invg[:, :msz],
                        op=mybir.AluOpType.mult,
                    )
                    j2 = lx * 2 + cx
                    nc.vector.tensor_tensor(
                        out=runmax[:, j2, :msz],
                        in0=runmax[:, j2, :msz],
                        in1=sim[:, :msz],
                        op=mybir.AluOpType.max,
                    )

            def layer_done(lx):
                # finalize this layer's local maxes while later layers compute
                for cx in range(2):
                    j = lx * 2 + cx
                    nc.vector.reduce_max(
                        lmax_sb[:, j : j + 1],
                        runmax[:, j, :],
                        axis=mybir.AxisListType.X,
                    )
                if not HOST_COMBINE and SPLIT_CC:
                    # per-layer 2-row all-reduce, overlapped with compute
                    nc.gpsimd.dma_start(
                        cc_in.ap()[2 * lx : 2 * lx + 2, :].rearrange("c p -> p c"),
                        lmax_sb[:, 2 * lx : 2 * lx + 2],
                    )
                    nc.gpsimd.collective_compute(
                        "AllReduce",
                        mybir.AluOpType.max,
                        replica_groups=[list(range(NCORES))],
                        ins=[cc_in.ap()[2 * lx : 2 * lx + 2, :].opt()],
                        outs=[cc_out.ap()[2 * lx : 2 * lx + 2, :].opt()],
                    )

            work = [
                (lx, m0, msz) for lx in range(L) for (m0, msz) in SUPERS
            ]
            pending = []
            done_lx = 0
            for sidx, (lx, m0, msz) in enumerate(work):
                pending.append((lx, m0, msz) + stage_a(lx, m0, msz, sidx))
                if len(pending) > SKEW:
                    blx = pending[0][0]
                    stage_b(*pending.pop(0))
                    nxt = pending[0][0] if pending else L
                    while done_lx < nxt:
                        layer_done(done_lx)
                        done_lx += 1
            while pending:
                blx = pending[0][0]
                stage_b(*pending.pop(0))
                nxt = pending[0][0] if pending else L
                while done_lx < nxt:
                    layer_done(done_lx)
                    done_lx += 1

            if HOST_COMBINE:
                # ship per-core local maxes and 1/||q||; host does the
                # 8-way max + mean + 1-x (the unshard/combine step)
                nc.sync.dma_start(lmax_ext.ap(), lmax_sb[:])
                nc.sync.dma_start(invq_ext.ap(), invq[:])
            else:
                if not SPLIT_CC:
                    nc.gpsimd.dma_start(
                        cc_in.ap().rearrange("c p -> p c"), lmax_sb[:]
                    )
                    nc.gpsimd.collective_compute(
                        "AllReduce",
                        mybir.AluOpType.max,
                        replica_groups=[list(range(NCORES))],
                        ins=[cc_in.ap().opt()],
                        outs=[cc_out.ap().opt()],
                    )
                nc.gpsimd.dma_start(
                    gmax_sb[:], cc_out.ap().rearrange("c p -> p c")
                )

                # ---- scale by 1/||q||, mean over layers, 1 - x ----
                nc.vector.tensor_tensor(
                    out=smax_sb[:],
                    in0=gmax_sb[:],
                    in1=invq[:],
                    op=mybir.AluOpType.mult,
                )
                for cx in range(2):
                    ssum = pp.tile([128, 1], F32, name=f"ssum{cx}")
                    nc.vector.reduce_sum(
                        ssum[:],
                        smax_sb[:, cx : 2 * L : 2],
                        axis=mybir.AxisListType.X,
                    )
                    # out = 1 - ssum/L
                    nc.scalar.activation(
                        res_sb[:, cx : cx + 1],
                        ssum[:],
                        AF.Copy,
                        bias=1.0,
                        scale=-1.0 / L,
                    )
                for cx in range(2):
                    nc.sync.dma_start(
                        out_ext.ap()[cx : cx + 1, :].rearrange("c p -> p c"),
                        res_sb[:, cx : cx + 1],
                    )

    nc.compile()
    return nc


_NC_CACHE = None


def _get_nc():
    global _NC_CACHE
    if _NC_CACHE is None:
        _NC_CACHE = build()
    return _NC_CACHE


def _prep_shard(g_lp, c):
    # [L, MS, D] slice -> [L, D, MS] contiguous
    sl = g_lp[:, c * MS : (c + 1) * MS, :]
    return np.ascontiguousarray(sl.transpose(0, 2, 1))


def _prep_inputs(test_patch_tokens, normal_patch_tokens):
    q = np.asarray(test_patch_tokens, dtype=np.float32)
    g = np.asarray(normal_patch_tokens, dtype=np.float32)
    qn_lp = q.astype(NP_IN)  # [L, N, D]
    qt_lp = np.ascontiguousarray(qn_lp.transpose(0, 2, 1))  # [L, D, N]
    g_lp = g.astype(NP_IN)  # [L, M, D]
    with ThreadPoolExecutor(NCORES) as ex:
        shards = list(ex.map(lambda c: _prep_shard(g_lp, c), range(NCORES)))
    return [
        {"g_t": shards[c], "q_t": qt_lp, "q_n": qn_lp} for c in range(NCORES)
    ]


def kernel(test_patch_tokens: np.ndarray, normal_patch_tokens: np.ndarray):
    in_maps = _prep_inputs(test_patch_tokens, normal_patch_tokens)
    nc = _get_nc()
    results = run_bass_kernel_spmd(nc, in_maps, core_ids=list(range(NCORES))).results
    if HOST_COMBINE:
        # combine per-shard partial results: global max over cores, then
        # 1/||q|| scale, mean over layers, 1-x (tiny: 8*[128,8] values)
        lmax = np.max(
            np.stack([results[c]["out_lmax"] for c in range(NCORES)]), axis=0
        )  # [128, 2L]: column j = layer*2 + chunk
        invq = results[0]["out_invq"]  # identical on all cores
        smax = lmax * invq
        test_sim = smax.reshape(128, L, 2).mean(axis=1)  # [128(p), 2(chunk)]
        out = 1.0 - test_sim.T.reshape(N)  # n = chunk*128 + p
        return out.astype(np.float32).reshape(1, 1, 16, 16)
    out = results[0]["out"].astype(np.float32).reshape(1, 1, 16, 16)
    return out


# revision 28
# speedup vs baseline: 1.5497x; 1.0157x over previous
"""Distributed Trainium2 kernel for AnomalyMoE k-NN retrieval.

reference:  q = l2norm(test[L,N,D]); g = l2norm(normal[L,M,D])
            sim[l,n,m] = q . g ; out = (1 - mean_l max_m sim).reshape(1,1,16,16)

Strategy (8 NeuronCores):
- Shard gallery along M (6400 rows/core). Host pre-transposes each shard to
  [L, D, M_shard] (fp8e4m3 by default) so every device DMA is dense; queries
  ship in both layouts ([L,D,N] for matmul weights, [L,N,D] for norms).
- Per core: dot[n,m] accumulated on TensorE with fp8 DoubleRow pair-matmuls
  (contraction 256/instruction).  Gallery row norms via ACT Square +
  ones-DoubleRow-matmul (broadcasts colsums to all 128 partitions), then
  one ACT Abs_reciprocal_sqrt.  Queries are NOT normalized on the way in:
  1/||q_n|| is applied to the per-layer maxes at the end (positive
  per-query scale commutes with max over gallery).
- local layer-max via DVE mult + running max; AllReduce(max) over 8 cores
  on [128, 8] f32; final mean over layers, 1-x, output [2,128] f32.
"""

import os
import sys
from concurrent.futures import ThreadPoolExecutor

sys.path.insert(0, "/opt/trn_rl_repo")

import numpy as np
import ml_dtypes

import concourse.bacc as bacc
import concourse.mybir as mybir
import concourse.tile as tile
from concourse.bass_utils import run_bass_kernel_spmd

F32 = mybir.dt.float32
BF16 = mybir.dt.bfloat16
AF = mybir.ActivationFunctionType
DR = mybir.MatmulPerfMode.DoubleRow

MODE = os.environ.get("KERNEL_MODE", "fp8")  # "fp8" | "bf16"
if MODE == "fp8":
    DT_IN = mybir.dt.float8e4
    NP_IN = ml_dtypes.float8_e4m3fn
else:
    DT_IN = mybir.dt.bfloat16
    NP_IN = ml_dtypes.bfloat16

NCORES = 8
L = 4
D = 1024
N = 256
M_FULL = 51200
MS = M_FULL // NCORES  # 6400 per core
KC = D // 128  # 8 contraction chunks of 128
KP = KC // 2  # 4 DoubleRow pairs
SUPER = 512
SUPERS = [(m0, min(SUPER, MS - m0)) for m0 in range(0, MS, SUPER)]  # 12x512 + 256
DVE_SQ_MOD = int(os.environ.get("KERNEL_DVE_SQ_MOD", "0"))  # 0 = all squares on ACT
SKEW = int(os.environ.get("KERNEL_SKEW", "1"))  # software-pipeline depth
NORM_DMA = os.environ.get("KERNEL_NORM_DMA", "0") == "1"
SQ_SPLIT = int(os.environ.get("KERNEL_SQ_SPLIT", "2"))  # k-chunks squared on DVE
HOST_COMBINE = os.environ.get("KERNEL_HOST_COMBINE", "0") == "1"
SPLIT_CC = os.environ.get("KERNEL_SPLIT_CC", "0") == "1"
MAX_GPSIMD = os.environ.get("KERNEL_MAX_GPSIMD", "0") == "1"
KERNEL_TAG = os.environ.get("KERNEL_TAG", "")
NEG = -3.0e38


def build():
    nc = bacc.Bacc("TRN2", target_bir_lowering=False, debug=False, num_devices=NCORES)
    g_ext = nc.dram_tensor("g_t", [L, D, MS], DT_IN, kind="ExternalInput")
    qt_ext = nc.dram_tensor("q_t", [L, D, N], DT_IN, kind="ExternalInput")
    qn_ext = nc.dram_tensor("q_n", [L, N, D], DT_IN, kind="ExternalInput")
    if HOST_COMBINE:
        lmax_ext = nc.dram_tensor("out_lmax", [128, 2 * L], F32, kind="ExternalOutput")
        invq_ext = nc.dram_tensor("out_invq", [128, 2 * L], F32, kind="ExternalOutput")
        out_ext = cc_in = cc_out = None
    else:
        out_ext = nc.dram_tensor("out", [2, 128], F32, kind="ExternalOutput")
        cc_in = nc.dram_tensor("cc_in", [2 * L, 128], F32)
        cc_out = nc.dram_tensor("cc_out", [2 * L, 128], F32, addr_space="Shared")
        lmax_ext = invq_ext = None

    with tile.TileContext(nc) as tc:
        with (
            tc.tile_pool(name="persist", bufs=1) as pp,
            tc.tile_pool(name="gsup", bufs=4) as gpool,
            tc.tile_pool(name="sqp", bufs=4) as sqpool,
            tc.tile_pool(name="invgp", bufs=3) as invgpool,
            tc.tile_pool(name="simp", bufs=3) as simpool,
            tc.tile_pool(name="qsqp", bufs=2) as qsqpool,
            tc.tile_pool(name="pm0", bufs=3, space="PSUM") as pm0pool,
            tc.tile_pool(name="pm1", bufs=3, space="PSUM") as pm1pool,
            tc.tile_pool(name="pnorm", bufs=2, space="PSUM") as pnormpool,
        ):
            # ---- persistent tiles ----
            qt_sb = pp.tile([128, L * KC, N], DT_IN, name="qt_sb")
            nc.sync.dma_start(
                qt_sb[:], qt_ext.ap().rearrange("l (k p) n -> p (l k) n", p=128)
            )
            qn_sb = pp.tile([128, 2 * L, D], DT_IN, name="qn_sb")
            if MODE == "fp8":
                ones_sb = pp.tile([128, 2, 128], DT_IN, name="ones_sb")
            else:
                ones_sb = pp.tile([128, 128], DT_IN, name="ones_sb")
            nc.gpsimd.memset(ones_sb[:], 1.0)
            runmax = pp.tile([128, 2 * L, SUPER], BF16, name="runmax")
            nc.gpsimd.memset(runmax[:], NEG)
            qss = pp.tile([128, 2 * L], F32, name="qss")
            invq = pp.tile([128, 2 * L], F32, name="invq")
            lmax_sb = pp.tile([128, 2 * L], F32, name="lmax_sb")
            gmax_sb = pp.tile([128, 2 * L], F32, name="gmax_sb")
            smax_sb = pp.tile([128, 2 * L], F32, name="smax_sb")
            res_sb = pp.tile([128, 2], F32, name="res_sb")
            if KERNEL_TAG:
                # cache-buster: changes the BIR so stale NEFF caches miss
                tag_sb = pp.tile([128, 1], F32, name=f"tag_{KERNEL_TAG}")
                nc.gpsimd.memset(tag_sb[:], 1.0)

            def emit_q_norms():
                # query squared norms: qss[:, l*2+c] = sum_d q[l, c*128+p, d]^2
                # (emitted after the main loop: only needed for the output)
                nc.sync.dma_start(
                    qn_sb[:], qn_ext.ap().rearrange("l (c p) d -> p (l c) d", p=128)
                )
                for j in range(2 * L):
                    qsq_scr = qsqpool.tile([128, D], BF16, name="qsq_scr")
                    nc.scalar.activation(
                        qsq_scr[:],
                        qn_sb[:, j, :],
                        AF.Square,
                        accum_out=qss[:, j : j + 1],
                    )
                # invq = 1/sqrt(qss)
                nc.scalar.activation(invq[:], qss[:], AF.Abs_reciprocal_sqrt)

            # ---- main loop over layers and m-supers, software-pipelined ----
            # Stage A (dma + squares) runs SKEW supers ahead of stage B
            # (norm-mm, rsqrt, main-mms, epilogue) so the in-order ACT queue
            # never head-of-line-blocks a square behind an rsqrt.

            def stage_a(lx, m0, msz, sidx):
                gsup = gpool.tile([128, KC, SUPER], DT_IN, name="gsup")
                nc.sync.dma_start(
                    gsup[:, :, :msz],
                    g_ext.ap()[lx].rearrange("(k p) m -> p k m", p=128)[
                        :, :, m0 : m0 + msz
                    ],
                )
                sq = sqpool.tile([128, KC, SUPER], DT_IN, name="sq")
                if SQ_SPLIT:
                    # fixed per-super split: last SQ_SPLIT k-chunks on DVE
                    ka = KC - SQ_SPLIT
                    nc.scalar.activation(
                        sq[:, : ka // 2, :msz], gsup[:, : ka // 2, :msz], AF.Square
                    )
                    nc.scalar.activation(
                        sq[:, ka // 2 : ka, :msz],
                        gsup[:, ka // 2 : ka, :msz],
                        AF.Square,
                    )
                    nc.vector.tensor_tensor(
                        out=sq[:, ka:, :msz],
                        in0=gsup[:, ka:, :msz],
                        in1=gsup[:, ka:, :msz],
                        op=mybir.AluOpType.mult,
                    )
                    return gsup, sq
                on_dve = DVE_SQ_MOD and sidx % DVE_SQ_MOD == DVE_SQ_MOD - 1
                for h in (0, 1):
                    ksl = slice(h * KC // 2, (h + 1) * KC // 2)
                    if on_dve:
                        nc.vector.tensor_tensor(
                            out=sq[:, ksl, :msz],
                            in0=gsup[:, ksl, :msz],
                            in1=gsup[:, ksl, :msz],
                            op=mybir.AluOpType.mult,
                        )
                    else:
                        nc.scalar.activation(
                            sq[:, ksl, :msz], gsup[:, ksl, :msz], AF.Square
                        )
                return gsup, sq

            def stage_b(lx, m0, msz, gsup, sq):
                # gallery norms: pnorm[p, m] = sum_d g[d, m]^2 (all rows equal)
                pnorm = pnormpool.tile([128, SUPER], F32, name="pnorm")
                if MODE == "fp8" and NORM_DMA:
                    # fold k-chunks 8->2 with CCE-add DMAs (off the PE), then
                    # a single DoubleRow ones-matmul does the partition sum.
                    nc.gpsimd.dma_start(
                        out=sq[:, 0:4, :msz],
                        in_=sq[:, 4:8, :msz],
                        accum_op=mybir.AluOpType.add,
                    )
                    nc.gpsimd.dma_start(
                        out=sq[:, 0:2, :msz],
                        in_=sq[:, 2:4, :msz],
                        accum_op=mybir.AluOpType.add,
                    )
                    nc.tensor.matmul(
                        pnorm[:, :msz],
                        ones_sb[:],
                        sq[:, 0:2, :msz],
                        start=True,
                        stop=True,
                        perf_mode=DR,
                    )
                elif MODE == "fp8":
                    for j in range(KP):
                        nc.tensor.matmul(
                            pnorm[:, :msz],
                            ones_sb[:],
                            sq[:, 2 * j : 2 * j + 2, :msz],
                            start=(j == 0),
                            stop=(j == KP - 1),
                            perf_mode=DR,
                        )
                else:
                    for k in range(KC):
                        nc.tensor.matmul(
                            pnorm[:, :msz],
                            ones_sb[:],
                            sq[:, k, :msz],
                            start=(k == 0),
                            stop=(k == KC - 1),
                        )
                # invg = 1/sqrt(pnorm) on ACT
                invg = invgpool.tile([128, SUPER], F32, name="invg")
                nc.scalar.activation(
                    invg[:, :msz], pnorm[:, :msz], AF.Abs_reciprocal_sqrt
                )

                for cx, pmpool in ((0, pm0pool), (1, pm1pool)):
                    pm = pmpool.tile([128, SUPER], F32, name=f"pm{cx}")
                    if MODE == "fp8":
                        for j in range(KP):
                            nc.tensor.matmul(
                                pm[:, :msz],
                                qt_sb[
                                    :,
                                    lx * KC + 2 * j : lx * KC + 2 * j + 2,
                                    cx * 128 : (cx + 1) * 128,
                                ],
                                gsup[:, 2 * j : 2 * j + 2, :msz],
                                start=(j == 0),
                                stop=(j == KP - 1),
                                perf_mode=DR,
                            )
                    else:
                        for k in range(KC):
                            nc.tensor.matmul(
                                pm[:, :msz],
                                qt_sb[:, lx * KC + k, cx * 128 : (cx + 1) * 128],
                                gsup[:, k, :msz],
                                start=(k == 0),
                                stop=(k == KC - 1),
                            )
                    sim = simpool.tile([128, SUPER], BF16, name="sim")
                    nc.vector.tensor_tensor(
                        out=sim[:, :msz],
                        in0=pm[:, :msz],
                        in1=invg[:, :msz],
                        op=mybir.AluOpType.mult,
                    )
                    j2 = lx * 2 + cx
                    max_eng = nc.gpsimd if MAX_GPSIMD else nc.vector
                    max_eng.tensor_tensor(
                        out=runmax[:, j2, :msz],
                        in0=runmax[:, j2, :msz],
                        in1=sim[:, :msz],
                        op=mybir.AluOpType.max,
                    )

            def layer_done(lx):
                # finalize this layer's local maxes while later layers compute
                for cx in range(2):
                    j = lx * 2 + cx
                    nc.vector.reduce_max(
                        lmax_sb[:, j : j + 1],
                        runmax[:, j, :],
                        axis=mybir.AxisListType.X,
                    )
                if not HOST_COMBINE and SPLIT_CC:
                    # per-layer 2-row all-reduce, overlapped with compute
                    nc.gpsimd.dma_start(
                        cc_in.ap()[2 * lx : 2 * lx + 2, :].rearrange("c p -> p c"),
                        lmax_sb[:, 2 * lx : 2 * lx + 2],
                    )
                    nc.gpsimd.collective_compute(
                        "AllReduce",
                        mybir.AluOpType.max,
                        replica_groups=[list(range(NCORES))],
                        ins=[cc_in.ap()[2 * lx : 2 * lx + 2, :].opt()],
                        outs=[cc_out.ap()[2 * lx : 2 * lx + 2, :].opt()],
                    )

            work = [
                (lx, m0, msz) for lx in range(L) for (m0, msz) in SUPERS
            ]
            pending = []
            done_lx = 0
            for sidx, (lx, m0, msz) in enumerate(work):
                pending.append((lx, m0, msz) + stage_a(lx, m0, msz, sidx))
                if len(pending) > SKEW:
                    blx = pending[0][0]
                    stage_b(*pending.pop(0))
                    nxt = pending[0][0] if pending else L
                    while done_lx < nxt:
                        layer_done(done_lx)
                        done_lx += 1
            while pending:
                blx = pending[0][0]
                stage_b(*pending.pop(0))
                nxt = pending[0][0] if pending else L
                while done_lx < nxt:
                    layer_done(done_lx)
                    done_lx += 1

            emit_q_norms()
            if HOST_COMBINE:
                # ship per-core local maxes and 1/||q||; host does the
                # 8-way max + mean + 1-x (the unshard/combine step)
                nc.sync.dma_start(lmax_ext.ap(), lmax_sb[:])
                nc.sync.dma_start(invq_ext.ap(), invq[:])
            else:
                if not SPLIT_CC:
                    nc.gpsimd.dma_start(
                        cc_in.ap().rearrange("c p -> p c"), lmax_sb[:]
                    )
                    nc.gpsimd.collective_compute(
                        "AllReduce",
                        mybir.AluOpType.max,
                        replica_groups=[list(range(NCORES))],
                        ins=[cc_in.ap().opt()],
                        outs=[cc_out.ap().opt()],
                    )
                nc.gpsimd.dma_start(
                    gmax_sb[:], cc_out.ap().rearrange("c p -> p c")
                )

                # ---- scale by 1/||q||, mean over layers, 1 - x ----
                nc.vector.tensor_tensor(
                    out=smax_sb[:],
                    in0=gmax_sb[:],
                    in1=invq[:],
                    op=mybir.AluOpType.mult,
                )
                for cx in range(2):
                    ssum = pp.tile([128, 1], F32, name=f"ssum{cx}")
                    nc.vector.reduce_sum(
                        ssum[:],
                        smax_sb[:, cx : 2 * L : 2],
                        axis=mybir.AxisListType.X,
                    )
                    # out = 1 - ssum/L
                    nc.scalar.activation(
                        res_sb[:, cx : cx + 1],
                        ssum[:],
                        AF.Copy,
                        bias=1.0,
                        scale=-1.0 / L,
                    )
                for cx in range(2):
                    nc.sync.dma_start(
                        out_ext.ap()[cx : cx + 1, :].rearrange("c p -> p c"),
                        res_sb[:, cx : cx + 1],
                    )

    nc.compile()
    return nc


_NC_CACHE = None


def _get_nc():
    global _NC_CACHE
    if _NC_CACHE is None:
        _NC_CACHE = build()
    return _NC_CACHE


def _prep_shard(g_lp, c):
    # [L, MS, D] slice -> [L, D, MS] contiguous
    sl = g_lp[:, c * MS : (c + 1) * MS, :]
    return np.ascontiguousarray(sl.transpose(0, 2, 1))


def _prep_inputs(test_patch_tokens, normal_patch_tokens):
    q = np.asarray(test_patch_tokens, dtype=np.float32)
    g = np.asarray(normal_patch_tokens, dtype=np.float32)
    qn_lp = q.astype(NP_IN)  # [L, N, D]
    qt_lp = np.ascontiguousarray(qn_lp.transpose(0, 2, 1))  # [L, D, N]
    g_lp = g.astype(NP_IN)  # [L, M, D]
    with ThreadPoolExecutor(NCORES) as ex:
        shards = list(ex.map(lambda c: _prep_shard(g_lp, c), range(NCORES)))
    return [
        {"g_t": shards[c], "q_t": qt_lp, "q_n": qn_lp} for c in range(NCORES)
    ]


def kernel(test_patch_tokens: np.ndarray, normal_patch_tokens: np.ndarray):
    in_maps = _prep_inputs(test_patch_tokens, normal_patch_tokens)
    nc = _get_nc()
    results = run_bass_kernel_spmd(nc, in_maps, core_ids=list(range(NCORES))).results
    if HOST_COMBINE:
        # combine per-shard partial results: global max over cores, then
        # 1/||q|| scale, mean over layers, 1-x (tiny: 8*[128,8] values)
        lmax = np.max(
            np.stack([results[c]["out_lmax"] for c in range(NCORES)]), axis=0
        )  # [128, 2L]: column j = layer*2 + chunk
        invq = results[0]["out_invq"]  # identical on all cores
        smax = lmax * invq
        test_sim = smax.reshape(128, L, 2).mean(axis=1)  # [128(p), 2(chunk)]
        out = 1.0 - test_sim.T.reshape(N)  # n = chunk*128 + p
        return out.astype(np.float32).reshape(1, 1, 16, 16)
    out = results[0]["out"].astype(np.float32).reshape(1, 1, 16, 16)
    return out
